# revision 5
# baseline (speedup 1.0000x reference)
"""Trainium2 Bass kernel for nn_NodeAggregator (gnn_message_passing).

Reference computation (per batch b):
    h      = relu(x @ W1 + b1)            [N, K]
    logits = h @ W2 + b2                  [N, K]
    (mask adds a per-row constant to logits -> softmax over K is invariant,
     so mask never affects the output and is ignored)
    S      = softmax(logits, axis=-1)     [N, K]
    pooled_x   = S^T @ x                  [K, F]
    pooled_adj = S^T @ adj @ S            [K, K]
    pmask  = ones[B, K]

Key restructuring: pooled_adj = (S^T @ adj) @ S, so we compute U = S^T @ adj
by streaming adj in natural row-major layout exactly once (memory-bound cost:
256 MB across 16 batches), then pooled_adj = U @ S.  `mid = adj @ S` is never
materialized.

Sharding: data-parallel over B across 8 cores (2 batches/core), params
replicated.  Everything is hardcoded for B=16, N=2048, F=128, K=64.
"""

import os
from contextlib import ExitStack

import numpy as np

import concourse.bacc as bacc
import concourse.tile as tile
from concourse import mybir
from concourse.bass_utils import run_bass_kernel_spmd
from concourse.masks import make_identity

B, N, F, K = 16, 2048, 128, 64
N_CORES = 8
B_PER = B // N_CORES          # batches per core
NCH = N // 128                # 16 chunks of 128 rows
F32 = mybir.dt.float32
F32R = mybir.dt.float32r
AFT = mybir.ActivationFunctionType

# fp32r runs the PE at 4x fp32 rate for the big S^T@adj matmul (free dim 512).
# Set BASS_U_F32R=0 to fall back to full fp32.
U_F32R = os.environ.get("BASS_U_F32R", "1") == "1"


def _body(ctx: ExitStack, tc: tile.TileContext, x, adj, W1, b1, W2, b2, px_out, pa_out):
    nc = tc.nc

    const = ctx.enter_context(tc.tile_pool(name="const", bufs=1))
    xnat_p = ctx.enter_context(tc.tile_pool(name="xnat", bufs=2))
    xt_p = ctx.enter_context(tc.tile_pool(name="xt", bufs=2))
    hta_p = ctx.enter_context(tc.tile_pool(name="hta", bufs=2))
    s_p = ctx.enter_context(tc.tile_pool(name="s", bufs=2))
    stat_p = ctx.enter_context(tc.tile_pool(name="stat", bufs=2))
    u_p = ctx.enter_context(tc.tile_pool(name="u", bufs=2))
    ut_p = ctx.enter_context(tc.tile_pool(name="ut", bufs=2))
    out_p = ctx.enter_context(tc.tile_pool(name="out", bufs=2))
    adj_p = ctx.enter_context(tc.tile_pool(name="adj", bufs=4))
    ps_small = ctx.enter_context(tc.tile_pool(name="ps_small", bufs=4, space="PSUM"))
    ps_u = ctx.enter_context(tc.tile_pool(name="ps_u", bufs=1, space="PSUM"))

    # Constants / params
    ident = const.tile([128, 128], F32)
    make_identity(nc, ident[:])
    w1_sb = const.tile([F, K], F32)
    nc.sync.dma_start(w1_sb[:], W1[:])
    b1_sb = const.tile([K, 1], F32)
    nc.sync.dma_start(b1_sb[:], b1[:])
    # [W2; b2] augmented so logits = [h | 1] @ [W2; b2] includes the bias.
    w2b2 = const.tile([K + 1, K], F32)
    nc.sync.dma_start(w2b2[0:K, :], W2[:])
    nc.sync.dma_start(w2b2[K : K + 1, :], b2[:])

    for b in range(B_PER):
        # ---- load x_b as 16 column-blocks of [128n x 128f] ----
        xnat = xnat_p.tile([128, NCH * F], F32)
        nc.sync.dma_start(
            xnat[:].rearrange("p (c f) -> p c f", f=F),
            x[b].rearrange("(c p) f -> p c f", p=128),
        )

        # ---- xT[f, n] via PE transposes ----
        xt = xt_p.tile([128, N], F32)
        for c in range(NCH):
            pt = ps_small.tile([128, 128], F32, tag="ps")
            nc.tensor.transpose(pt[:], xnat[:, c * 128 : (c + 1) * 128], ident[:])
            nc.scalar.copy(xt[:, c * 128 : (c + 1) * 128], pt[:])

        # ---- hT = relu(W1^T @ xT + b1), augmented with a row of ones ----
        hta = hta_p.tile([K + 1, N], F32)
        nc.gpsimd.memset(hta[K : K + 1, :], 1.0)
        for s in range(4):
            ph = ps_small.tile([K, 512], F32, tag="ps")
            nc.tensor.matmul(
                ph[:], w1_sb[:], xt[:, s * 512 : (s + 1) * 512], start=True, stop=True
            )
            nc.scalar.activation(
                hta[0:K, s * 512 : (s + 1) * 512], ph[:], AFT.Relu, bias=b1_sb[:]
            )

        # ---- logits chunks -> exp -> row sums ----
        S = s_p.tile([128, NCH * K], F32)
        ssum = stat_p.tile([128, NCH], F32, tag="ssum")
        rinv = stat_p.tile([128, NCH], F32, tag="rinv")
        for c in range(NCH):
            pl = ps_small.tile([128, K], F32, tag="ps")
            nc.tensor.matmul(
                pl[:], hta[:, c * 128 : (c + 1) * 128], w2b2[:], start=True, stop=True
            )
            nc.scalar.activation(
                S[:, c * K : (c + 1) * K], pl[:], AFT.Exp,
                accum_out=ssum[:, c : c + 1],
            )
        nc.vector.reciprocal(rinv[:], ssum[:])
        for c in range(NCH):
            nc.vector.tensor_scalar_mul(
                S[:, c * K : (c + 1) * K], S[:, c * K : (c + 1) * K], rinv[:, c : c + 1]
            )
        if U_F32R:
            # fp32r copy of S for the big matmul (PE rounds fp32r on read;
            # the verifier wants operands produced as fp32r)
            s_r = s_p.tile([128, NCH * K], F32R, tag="s_r")
            nc.vector.tensor_copy(s_r[:], S[:])

        # ---- pooled_x = S^T @ x ----
        ppx = ps_small.tile([K, F], F32, tag="ps")
        for c in range(NCH):
            nc.tensor.matmul(
                ppx[:],
                S[:, c * K : (c + 1) * K],
                xnat[:, c * 128 : (c + 1) * 128],
                start=(c == 0),
                stop=(c == NCH - 1),
            )
        px_sb = out_p.tile([K, F], F32, tag="px")
        nc.scalar.copy(px_sb[:], ppx[:])
        nc.sync.dma_start(px_out[b], px_sb[:])

        # ---- U = S^T @ adj  (the big memory-bound stream over adj) ----
        pus = [ps_u.tile([K, 512], F32, tag=f"pu{s}", name=f"pu_{b}_{s}") for s in range(4)]
        adt = F32R if U_F32R else F32
        for c in range(NCH):
            at = adj_p.tile([128, N], adt)
            src = adj[b, c * 128 : (c + 1) * 128, :]
            if U_F32R:
                src = src.bitcast(F32R)
            nc.sync.dma_start(at[:], src)
            for s in range(4):
                lhs = s_r[:, c * K : (c + 1) * K] if U_F32R else S[:, c * K : (c + 1) * K]
                nc.tensor.matmul(
                    pus[s][:], lhs, at[:, s * 512 : (s + 1) * 512],
                    start=(c == 0), stop=(c == NCH - 1),
                )
        u_sb = u_p.tile([K, N], F32)
        for s in range(4):
            nc.scalar.copy(u_sb[:, s * 512 : (s + 1) * 512], pus[s][:])

        # ---- UT via PE transposes ----
        ut = ut_p.tile([128, NCH * K], F32)
        for c in range(NCH):
            pt2 = ps_small.tile([128, K], F32, tag="ps")
            nc.tensor.transpose(
                pt2[:], u_sb[:, c * 128 : (c + 1) * 128], ident[0:K, 0:K]
            )
            nc.vector.tensor_copy(ut[:, c * K : (c + 1) * K], pt2[:])

        # ---- pooled_adj = U @ S ----
        ppa = ps_small.tile([K, K], F32, tag="ps")
        for c in range(NCH):
            nc.tensor.matmul(
                ppa[:],
                ut[:, c * K : (c + 1) * K],
                S[:, c * K : (c + 1) * K],
                start=(c == 0),
                stop=(c == NCH - 1),
            )
        pa_sb = out_p.tile([K, K], F32, tag="pa")
        nc.scalar.copy(pa_sb[:], ppa[:])
        nc.sync.dma_start(pa_out[b], pa_sb[:])


_cached_nc = None


def _build():
    global _cached_nc
    if _cached_nc is not None:
        return _cached_nc
    nc = bacc.Bacc("TRN2", target_bir_lowering=False, debug=False)
    x = nc.dram_tensor("x", [B_PER, N, F], F32, kind="ExternalInput").ap()
    adj = nc.dram_tensor("adj", [B_PER, N, N], F32, kind="ExternalInput").ap()
    W1 = nc.dram_tensor("W1", [F, K], F32, kind="ExternalInput").ap()
    b1 = nc.dram_tensor("b1", [K, 1], F32, kind="ExternalInput").ap()
    W2 = nc.dram_tensor("W2", [K, K], F32, kind="ExternalInput").ap()
    b2 = nc.dram_tensor("b2", [1, K], F32, kind="ExternalInput").ap()
    px_out = nc.dram_tensor("pooled_x", [B_PER, K, F], F32, kind="ExternalOutput").ap()
    pa_out = nc.dram_tensor("pooled_adj", [B_PER, K, K], F32, kind="ExternalOutput").ap()
    with tile.TileContext(nc) as tc, ExitStack() as ctx:
        _body(ctx, tc, x, adj, W1, b1, W2, b2, px_out, pa_out)
    nc.compile()
    _cached_nc = nc
    return nc


def _in_maps(x, adj, W1, b1, W2, b2):
    x = np.ascontiguousarray(x, dtype=np.float32)
    adj = np.ascontiguousarray(adj, dtype=np.float32)
    maps = []
    for i in range(N_CORES):
        lo, hi = i * B_PER, (i + 1) * B_PER
        maps.append(
            {
                "x": x[lo:hi],
                "adj": adj[lo:hi],
                "W1": np.ascontiguousarray(W1, dtype=np.float32),
                "b1": np.ascontiguousarray(b1, dtype=np.float32).reshape(K, 1),
                "W2": np.ascontiguousarray(W2, dtype=np.float32),
                "b2": np.ascontiguousarray(b2, dtype=np.float32).reshape(1, K),
            }
        )
    return maps


def kernel(x, adj, mask, W1, b1, W2, b2):
    nc = _build()
    res = run_bass_kernel_spmd(nc, _in_maps(x, adj, W1, b1, W2, b2), list(range(N_CORES)))
    pooled_x = np.concatenate([r["pooled_x"] for r in res.results], axis=0)
    pooled_adj = np.concatenate([r["pooled_adj"] for r in res.results], axis=0)
    pmask = np.ones((B, K), dtype=np.float32)
    return pooled_x, pooled_adj, pmask


# revision 6
# speedup vs baseline: 6.4691x; 6.4691x over previous
"""Trainium2 Bass kernel for nn_NodeAggregator (gnn_message_passing).

Reference computation (per batch b):
    h      = relu(x @ W1 + b1)            [N, K]
    logits = h @ W2 + b2                  [N, K]
    (mask adds a per-row constant to logits -> softmax over K is invariant,
     so mask never affects the output and is ignored)
    S      = softmax(logits, axis=-1)     [N, K]
    pooled_x   = S^T @ x                  [K, F]
    pooled_adj = S^T @ adj @ S            [K, K]
    pmask  = ones[B, K]

Key restructuring: pooled_adj = (S^T @ adj) @ S, so we compute U = S^T @ adj
by streaming adj in natural row-major layout exactly once (memory-bound cost:
256 MB across 16 batches), then pooled_adj = U @ S.  `mid = adj @ S` is never
materialized.

Sharding: data-parallel over B across 8 cores (2 batches/core), params
replicated.  Everything is hardcoded for B=16, N=2048, F=128, K=64.
"""

import os
from contextlib import ExitStack

import numpy as np

import concourse.bacc as bacc
import concourse.tile as tile
from concourse import mybir
from concourse.bass_utils import run_bass_kernel_spmd
from concourse.masks import make_identity

B, N, F, K = 16, 2048, 128, 64
N_CORES = 8
B_PER = B // N_CORES          # batches per core
NCH = N // 128                # 16 chunks of 128 rows
F32 = mybir.dt.float32
F32R = mybir.dt.float32r
AFT = mybir.ActivationFunctionType

# fp32r runs the PE at 4x fp32 rate for the big S^T@adj matmul (free dim 512).
# Set BASS_U_F32R=0 to fall back to full fp32.
U_F32R = os.environ.get("BASS_U_F32R", "1") == "1"


def _body(ctx: ExitStack, tc: tile.TileContext, x, adj, W1, b1, W2, b2, px_out, pa_out,
          repeats=1):
    nc = tc.nc

    const = ctx.enter_context(tc.tile_pool(name="const", bufs=1))
    xnat_p = ctx.enter_context(tc.tile_pool(name="xnat", bufs=2))
    xt_p = ctx.enter_context(tc.tile_pool(name="xt", bufs=2))
    hta_p = ctx.enter_context(tc.tile_pool(name="hta", bufs=2))
    s_p = ctx.enter_context(tc.tile_pool(name="s", bufs=2))
    stat_p = ctx.enter_context(tc.tile_pool(name="stat", bufs=2))
    u_p = ctx.enter_context(tc.tile_pool(name="u", bufs=2))
    ut_p = ctx.enter_context(tc.tile_pool(name="ut", bufs=2))
    out_p = ctx.enter_context(tc.tile_pool(name="out", bufs=2))
    adj_p = ctx.enter_context(tc.tile_pool(name="adj", bufs=4))
    ps_small = ctx.enter_context(tc.tile_pool(name="ps_small", bufs=4, space="PSUM"))
    ps_u = ctx.enter_context(tc.tile_pool(name="ps_u", bufs=1, space="PSUM"))

    # Constants / params
    ident = const.tile([128, 128], F32)
    make_identity(nc, ident[:])
    w1_sb = const.tile([F, K], F32)
    nc.sync.dma_start(w1_sb[:], W1[:])
    b1_sb = const.tile([K, 1], F32)
    nc.sync.dma_start(b1_sb[:], b1[:])
    # [W2; b2] augmented so logits = [h | 1] @ [W2; b2] includes the bias.
    w2b2 = const.tile([K + 1, K], F32)
    nc.sync.dma_start(w2b2[0:K, :], W2[:])
    nc.sync.dma_start(w2b2[K : K + 1, :], b2[:])

    for b in [b for _ in range(repeats) for b in range(B_PER)]:
        # ---- load x_b as 16 column-blocks of [128n x 128f] ----
        xnat = xnat_p.tile([128, NCH * F], F32)
        nc.sync.dma_start(
            xnat[:].rearrange("p (c f) -> p c f", f=F),
            x[b].rearrange("(c p) f -> p c f", p=128),
        )

        # ---- xT[f, n] via PE transposes ----
        xt = xt_p.tile([128, N], F32)
        for c in range(NCH):
            pt = ps_small.tile([128, 128], F32, tag="ps")
            nc.tensor.transpose(pt[:], xnat[:, c * 128 : (c + 1) * 128], ident[:])
            nc.scalar.copy(xt[:, c * 128 : (c + 1) * 128], pt[:])

        # ---- hT = relu(W1^T @ xT + b1), augmented with a row of ones ----
        hta = hta_p.tile([K + 1, N], F32)
        nc.gpsimd.memset(hta[K : K + 1, :], 1.0)
        for s in range(4):
            ph = ps_small.tile([K, 512], F32, tag="ps")
            nc.tensor.matmul(
                ph[:], w1_sb[:], xt[:, s * 512 : (s + 1) * 512], start=True, stop=True
            )
            nc.scalar.activation(
                hta[0:K, s * 512 : (s + 1) * 512], ph[:], AFT.Relu, bias=b1_sb[:]
            )

        # ---- logits chunks -> exp -> row sums ----
        S = s_p.tile([128, NCH * K], F32)
        ssum = stat_p.tile([128, NCH], F32, tag="ssum")
        rinv = stat_p.tile([128, NCH], F32, tag="rinv")
        for c in range(NCH):
            pl = ps_small.tile([128, K], F32, tag="ps")
            nc.tensor.matmul(
                pl[:], hta[:, c * 128 : (c + 1) * 128], w2b2[:], start=True, stop=True
            )
            nc.scalar.activation(
                S[:, c * K : (c + 1) * K], pl[:], AFT.Exp,
                accum_out=ssum[:, c : c + 1],
            )
        nc.vector.reciprocal(rinv[:], ssum[:])
        for c in range(NCH):
            nc.vector.tensor_scalar_mul(
                S[:, c * K : (c + 1) * K], S[:, c * K : (c + 1) * K], rinv[:, c : c + 1]
            )
        if U_F32R:
            # fp32r copy of S for the big matmul (PE rounds fp32r on read;
            # the verifier wants operands produced as fp32r)
            s_r = s_p.tile([128, NCH * K], F32R, tag="s_r")
            nc.vector.tensor_copy(s_r[:], S[:])

        # ---- pooled_x = S^T @ x ----
        ppx = ps_small.tile([K, F], F32, tag="ps")
        for c in range(NCH):
            nc.tensor.matmul(
                ppx[:],
                S[:, c * K : (c + 1) * K],
                xnat[:, c * 128 : (c + 1) * 128],
                start=(c == 0),
                stop=(c == NCH - 1),
            )
        px_sb = out_p.tile([K, F], F32, tag="px")
        nc.scalar.copy(px_sb[:], ppx[:])
        nc.sync.dma_start(px_out[b], px_sb[:])

        # ---- U = S^T @ adj  (the big memory-bound stream over adj) ----
        pus = [ps_u.tile([K, 512], F32, tag=f"pu{s}", name=f"pu_{b}_{s}_{nc.next_id()}") for s in range(4)]
        adt = F32R if U_F32R else F32
        for c in range(NCH):
            at = adj_p.tile([128, N], adt)
            src = adj[b, c * 128 : (c + 1) * 128, :]
            if U_F32R:
                src = src.bitcast(F32R)
            nc.sync.dma_start(at[:], src)
            for s in range(4):
                lhs = s_r[:, c * K : (c + 1) * K] if U_F32R else S[:, c * K : (c + 1) * K]
                nc.tensor.matmul(
                    pus[s][:], lhs, at[:, s * 512 : (s + 1) * 512],
                    start=(c == 0), stop=(c == NCH - 1),
                )
        u_sb = u_p.tile([K, N], F32)
        for s in range(4):
            nc.scalar.copy(u_sb[:, s * 512 : (s + 1) * 512], pus[s][:])

        # ---- UT via PE transposes ----
        ut = ut_p.tile([128, NCH * K], F32)
        for c in range(NCH):
            pt2 = ps_small.tile([128, K], F32, tag="ps")
            nc.tensor.transpose(
                pt2[:], u_sb[:, c * 128 : (c + 1) * 128], ident[0:K, 0:K]
            )
            nc.vector.tensor_copy(ut[:, c * K : (c + 1) * K], pt2[:])

        # ---- pooled_adj = U @ S ----
        ppa = ps_small.tile([K, K], F32, tag="ps")
        for c in range(NCH):
            nc.tensor.matmul(
                ppa[:],
                ut[:, c * K : (c + 1) * K],
                S[:, c * K : (c + 1) * K],
                start=(c == 0),
                stop=(c == NCH - 1),
            )
        pa_sb = out_p.tile([K, K], F32, tag="pa")
        nc.scalar.copy(pa_sb[:], ppa[:])
        nc.sync.dma_start(pa_out[b], pa_sb[:])


_cached_nc = {}


def _build(repeats=1):
    if repeats in _cached_nc:
        return _cached_nc[repeats]
    nc = bacc.Bacc("TRN2", target_bir_lowering=False, debug=False)
    x = nc.dram_tensor("x", [B_PER, N, F], F32, kind="ExternalInput").ap()
    adj = nc.dram_tensor("adj", [B_PER, N, N], F32, kind="ExternalInput").ap()
    W1 = nc.dram_tensor("W1", [F, K], F32, kind="ExternalInput").ap()
    b1 = nc.dram_tensor("b1", [K, 1], F32, kind="ExternalInput").ap()
    W2 = nc.dram_tensor("W2", [K, K], F32, kind="ExternalInput").ap()
    b2 = nc.dram_tensor("b2", [1, K], F32, kind="ExternalInput").ap()
    px_out = nc.dram_tensor("pooled_x", [B_PER, K, F], F32, kind="ExternalOutput").ap()
    pa_out = nc.dram_tensor("pooled_adj", [B_PER, K, K], F32, kind="ExternalOutput").ap()
    with tile.TileContext(nc) as tc, ExitStack() as ctx:
        _body(ctx, tc, x, adj, W1, b1, W2, b2, px_out, pa_out, repeats=repeats)
    nc.compile()
    _cached_nc[repeats] = nc
    return nc


def _in_maps(x, adj, W1, b1, W2, b2):
    x = np.ascontiguousarray(x, dtype=np.float32)
    adj = np.ascontiguousarray(adj, dtype=np.float32)
    maps = []
    for i in range(N_CORES):
        lo, hi = i * B_PER, (i + 1) * B_PER
        maps.append(
            {
                "x": x[lo:hi],
                "adj": adj[lo:hi],
                "W1": np.ascontiguousarray(W1, dtype=np.float32),
                "b1": np.ascontiguousarray(b1, dtype=np.float32).reshape(K, 1),
                "W2": np.ascontiguousarray(W2, dtype=np.float32),
                "b2": np.ascontiguousarray(b2, dtype=np.float32).reshape(1, K),
            }
        )
    return maps


def kernel(x, adj, mask, W1, b1, W2, b2):
    nc = _build()
    res = run_bass_kernel_spmd(nc, _in_maps(x, adj, W1, b1, W2, b2), list(range(N_CORES)))
    pooled_x = np.concatenate([r["pooled_x"] for r in res.results], axis=0)
    pooled_adj = np.concatenate([r["pooled_adj"] for r in res.results], axis=0)
    pmask = np.ones((B, K), dtype=np.float32)
    return pooled_x, pooled_adj, pmask


# revision 10
# speedup vs baseline: 90.2067x; 13.9443x over previous
"""Trainium2 Bass kernel for nn_NodeAggregator (gnn_message_passing).

Reference computation (per batch b):
    h      = relu(x @ W1 + b1)            [N, K]
    logits = h @ W2 + b2                  [N, K]
    (mask adds a per-row constant to logits -> softmax over K is invariant,
     so mask never affects the output and is ignored)
    S      = softmax(logits, axis=-1)     [N, K]
    pooled_x   = S^T @ x                  [K, F]
    pooled_adj = S^T @ adj @ S            [K, K]
    pmask  = ones[B, K]

Key restructuring: pooled_adj = (S^T @ adj) @ S, so we compute U = S^T @ adj
by streaming adj in natural row-major layout exactly once (the memory-bound
cost: 256 MB across 16 batches), then pooled_adj = U @ S.  `mid = adj @ S`
is never materialized.

Schedule: per-batch "front" (x load, x^T, h^T, softmax, pooled_x) for ALL
batches first, then the per-batch adj streams back-to-back.  The adj stream
owns the nc.sync DMA queue exclusively; all small DMAs go on other engine
queues to avoid head-of-line blocking of the stream.

Sharding: data-parallel over B across 8 cores (2 batches/core), params
replicated.  Everything is hardcoded for B=16, N=2048, F=128, K=64.
"""

import os
from contextlib import ExitStack

import numpy as np

import concourse.bacc as bacc
import concourse.tile as tile
from concourse import mybir
from concourse.bass_utils import run_bass_kernel_spmd
from concourse.masks import make_identity

B, N, F, K = 16, 2048, 128, 64
N_CORES = 8
B_PER = B // N_CORES          # batches per core
NCH = N // 128                # 16 chunks of 128 rows
F32 = mybir.dt.float32
F32R = mybir.dt.float32r
AFT = mybir.ActivationFunctionType

# fp32r runs the PE at 4x fp32 rate for the big S^T@adj matmul (free dim 512).
# Set BASS_U_F32R=0 to fall back to full fp32.
U_F32R = os.environ.get("BASS_U_F32R", "1") == "1"
ADJ_BUFS = int(os.environ.get("BASS_ADJ_BUFS", "6"))
GSZ = int(os.environ.get("BASS_GSZ", "2"))  # 128-row chunks per adj DMA


def _front(nc, pools, consts, b, x, px_out):
    """x load -> x^T -> h^T -> softmax -> pooled_x.  Returns (S, S_r)."""
    ident, w1_sb, b1_sb, w2b2 = consts

    xnat = pools["xnat"].tile([128, NCH * F], F32, name=f"xnat_{b}", tag="xnat")
    nc.gpsimd.dma_start(
        xnat[:].rearrange("p (c f) -> p c f", f=F),
        x[b].rearrange("(c p) f -> p c f", p=128),
    )

    xt = pools["xt"].tile([128, N], F32, name=f"xt_{b}", tag="xt")
    for c in range(NCH):
        pt = pools["ps"].tile([128, 128], F32, tag="ps", name=f"pt_{b}_{c}")
        nc.tensor.transpose(pt[:], xnat[:, c * 128 : (c + 1) * 128], ident[:])
        nc.scalar.copy(xt[:, c * 128 : (c + 1) * 128], pt[:])

    hta = pools["hta"].tile([K + 1, N], F32, name=f"hta_{b}", tag="hta")
    nc.gpsimd.memset(hta[K : K + 1, :], 1.0)
    for s in range(4):
        ph = pools["ps"].tile([K, 512], F32, tag="ps", name=f"ph_{b}_{s}")
        nc.tensor.matmul(
            ph[:], w1_sb[:], xt[:, s * 512 : (s + 1) * 512], start=True, stop=True
        )
        nc.scalar.activation(
            hta[0:K, s * 512 : (s + 1) * 512], ph[:], AFT.Relu, bias=b1_sb[:]
        )

    S = pools["s"].tile([128, NCH * K], F32, name=f"S_{b}", tag="S")
    ssum = pools["stat"].tile([128, NCH], F32, tag="ssum", name=f"ssum_{b}")
    rinv = pools["stat"].tile([128, NCH], F32, tag="rinv", name=f"rinv_{b}")
    for c in range(NCH):
        pl = pools["ps"].tile([128, K], F32, tag="ps", name=f"pl_{b}_{c}")
        nc.tensor.matmul(
            pl[:], hta[:, c * 128 : (c + 1) * 128], w2b2[:], start=True, stop=True
        )
        nc.scalar.activation(
            S[:, c * K : (c + 1) * K], pl[:], AFT.Exp, accum_out=ssum[:, c : c + 1]
        )
    nc.vector.reciprocal(rinv[:], ssum[:])
    for c in range(NCH):
        nc.vector.tensor_scalar_mul(
            S[:, c * K : (c + 1) * K], S[:, c * K : (c + 1) * K], rinv[:, c : c + 1]
        )
    s_r = None
    if U_F32R:
        s_r = pools["s"].tile([128, NCH * K], F32R, name=f"s_r_{b}", tag="s_r")
        nc.vector.tensor_copy(s_r[:], S[:])

    # pooled_x = S^T @ x
    ppx = pools["ps"].tile([K, F], F32, tag="ps", name=f"ppx_{b}")
    for c in range(NCH):
        nc.tensor.matmul(
            ppx[:],
            S[:, c * K : (c + 1) * K],
            xnat[:, c * 128 : (c + 1) * 128],
            start=(c == 0),
            stop=(c == NCH - 1),
        )
    px_sb = pools["out"].tile([K, F], F32, tag="px", name=f"px_sb_{b}")
    nc.scalar.copy(px_sb[:], ppx[:])
    nc.scalar.dma_start(px_out[b], px_sb[:])
    return S, s_r


def _back(nc, pools, consts, b, adj, S, s_r, pa_out):
    """U = S^T @ adj streamed over adj, then pooled_adj = U @ S.

    adj streams as GSZ-row-chunk (GSZ MB) DMAs: larger transfers measure much
    faster than 1MB on HW (566 vs 332 GB/s/core).
    """
    ident = consts[0]
    pus = [
        pools["ps_u"].tile([K, 512], F32, tag=f"pu{s}", name=f"pu_{b}_{s}_{nc.next_id()}")
        for s in range(4)
    ]
    adt = F32R if U_F32R else F32
    for t in range(NCH // GSZ):
        at = pools["adj"].tile(
            [128, GSZ * N], adt, tag="at", name=f"at_{b}_{t}_{nc.next_id()}"
        )
        src = adj[b, t * GSZ * 128 : (t + 1) * GSZ * 128, :]
        if U_F32R:
            src = src.bitcast(F32R)
        nc.sync.dma_start(
            at[:].rearrange("p (g m) -> p g m", g=GSZ),
            src.rearrange("(g p) m -> p g m", p=128),
        )
        for g in range(GSZ):
            c = t * GSZ + g
            for s in range(4):
                lhs = s_r[:, c * K : (c + 1) * K] if U_F32R else S[:, c * K : (c + 1) * K]
                nc.tensor.matmul(
                    pus[s][:], lhs, at[:, g * N + s * 512 : g * N + (s + 1) * 512],
                    start=(c == 0), stop=(c == NCH - 1),
                )

    # tail: per stripe, copy U out of PSUM, transpose its 4 column blocks,
    # and feed the pooled_adj accumulation immediately.
    u_sb = pools["u"].tile([K, N], F32, name=f"u_sb_{b}", tag="u")
    ut = pools["ut"].tile([128, NCH * K], F32, name=f"ut_{b}", tag="ut")
    ppa = pools["ps"].tile([K, K], F32, tag="ps", name=f"ppa_{b}")
    for s in range(4):
        nc.scalar.copy(u_sb[:, s * 512 : (s + 1) * 512], pus[s][:])
        for j in range(4):
            c = s * 4 + j
            pt2 = pools["ps"].tile([128, K], F32, tag="ps", name=f"pt2_{b}_{c}")
            nc.tensor.transpose(
                pt2[:], u_sb[:, c * 128 : (c + 1) * 128], ident[0:K, 0:K]
            )
            cp = nc.vector.tensor_copy if j % 2 == 0 else nc.scalar.copy
            cp(ut[:, c * K : (c + 1) * K], pt2[:])
            nc.tensor.matmul(
                ppa[:],
                ut[:, c * K : (c + 1) * K],
                S[:, c * K : (c + 1) * K],
                start=(c == 0),
                stop=(c == NCH - 1),
            )
    pa_sb = pools["out"].tile([K, K], F32, tag="pa", name=f"pa_sb_{b}")
    nc.scalar.copy(pa_sb[:], ppa[:])
    nc.scalar.dma_start(pa_out[b], pa_sb[:])


def _body(ctx: ExitStack, tc: tile.TileContext, x, adj, W1, b1, W2, b2, px_out, pa_out,
          repeats=1):
    nc = tc.nc

    pools = {
        "xnat": ctx.enter_context(tc.tile_pool(name="xnat", bufs=2)),
        "xt": ctx.enter_context(tc.tile_pool(name="xt", bufs=2)),
        "hta": ctx.enter_context(tc.tile_pool(name="hta", bufs=2)),
        "s": ctx.enter_context(tc.tile_pool(name="s", bufs=2)),
        "stat": ctx.enter_context(tc.tile_pool(name="stat", bufs=2)),
        "u": ctx.enter_context(tc.tile_pool(name="u", bufs=2)),
        "ut": ctx.enter_context(tc.tile_pool(name="ut", bufs=2)),
        "out": ctx.enter_context(tc.tile_pool(name="out", bufs=2)),
        "adj": ctx.enter_context(tc.tile_pool(name="adj", bufs=ADJ_BUFS)),
        "ps": ctx.enter_context(tc.tile_pool(name="ps_small", bufs=4, space="PSUM")),
        "ps_u": ctx.enter_context(tc.tile_pool(name="ps_u", bufs=1, space="PSUM")),
    }
    const = ctx.enter_context(tc.tile_pool(name="const", bufs=1))

    ident = const.tile([128, 128], F32)
    make_identity(nc, ident[:])
    w1_sb = const.tile([F, K], F32)
    nc.gpsimd.dma_start(w1_sb[:], W1[:])
    b1_sb = const.tile([K, 1], F32)
    nc.gpsimd.dma_start(b1_sb[:], b1[:])
    # [W2; b2] augmented so logits = [h | 1] @ [W2; b2] includes the bias.
    w2b2 = const.tile([K + 1, K], F32)
    nc.gpsimd.dma_start(w2b2[0:K, :], W2[:])
    nc.gpsimd.dma_start(w2b2[K : K + 1, :], b2[:])
    consts = (ident, w1_sb, b1_sb, w2b2)

    for _ in range(repeats):
        fronts = [_front(nc, pools, consts, b, x, px_out) for b in range(B_PER)]
        for b in range(B_PER):
            S, s_r = fronts[b]
            _back(nc, pools, consts, b, adj, S, s_r, pa_out)


_cached_nc = {}


def _build(repeats=1):
    if repeats in _cached_nc:
        return _cached_nc[repeats]
    nc = bacc.Bacc("TRN2", target_bir_lowering=False, debug=False)
    x = nc.dram_tensor("x", [B_PER, N, F], F32, kind="ExternalInput").ap()
    adj = nc.dram_tensor("adj", [B_PER, N, N], F32, kind="ExternalInput").ap()
    W1 = nc.dram_tensor("W1", [F, K], F32, kind="ExternalInput").ap()
    b1 = nc.dram_tensor("b1", [K, 1], F32, kind="ExternalInput").ap()
    W2 = nc.dram_tensor("W2", [K, K], F32, kind="ExternalInput").ap()
    b2 = nc.dram_tensor("b2", [1, K], F32, kind="ExternalInput").ap()
    px_out = nc.dram_tensor("pooled_x", [B_PER, K, F], F32, kind="ExternalOutput").ap()
    pa_out = nc.dram_tensor("pooled_adj", [B_PER, K, K], F32, kind="ExternalOutput").ap()
    with tile.TileContext(nc) as tc, ExitStack() as ctx:
        _body(ctx, tc, x, adj, W1, b1, W2, b2, px_out, pa_out, repeats=repeats)
    nc.compile()
    _cached_nc[repeats] = nc
    return nc


def _in_maps(x, adj, W1, b1, W2, b2):
    x = np.ascontiguousarray(x, dtype=np.float32)
    adj = np.ascontiguousarray(adj, dtype=np.float32)
    maps = []
    for i in range(N_CORES):
        lo, hi = i * B_PER, (i + 1) * B_PER
        maps.append(
            {
                "x": x[lo:hi],
                "adj": adj[lo:hi],
                "W1": np.ascontiguousarray(W1, dtype=np.float32),
                "b1": np.ascontiguousarray(b1, dtype=np.float32).reshape(K, 1),
                "W2": np.ascontiguousarray(W2, dtype=np.float32),
                "b2": np.ascontiguousarray(b2, dtype=np.float32).reshape(1, K),
            }
        )
    return maps


def kernel(x, adj, mask, W1, b1, W2, b2):
    nc = _build()
    res = run_bass_kernel_spmd(nc, _in_maps(x, adj, W1, b1, W2, b2), list(range(N_CORES)))
    pooled_x = np.concatenate([r["pooled_x"] for r in res.results], axis=0)
    pooled_adj = np.concatenate([r["pooled_adj"] for r in res.results], axis=0)
    pmask = np.ones((B, K), dtype=np.float32)
    return pooled_x, pooled_adj, pmask
